# revision 32
# baseline (speedup 1.0000x reference)
"""Trainium2 Bass kernel for a 2-layer GCN (NextHopGNN).

Distribution: destination-node sharding across 8 NeuronCores. Each core owns
N/8 destination nodes and all edges pointing into them. Per layer:
  1. each core computes its slice of the scaled feature table
     y = dinv * (x @ W)  (PE matmuls, dinv = 1/sqrt(degree) with self-loops)
  2. AllGather -> every core holds the full [N, 64] table in HBM
  3. per 128-node dst tile: dma_gather the edge source rows (256B rows,
     int16 indices relative to one of 4 source blocks), build one-hot
     scatter matrices on the vector engine (batched iota == dstloc), and
     accumulate S^T @ G in PSUM on the tensor engine. Self-loop added via an
     identity matmul, bias via a rank-1 matmul pre-multiplied by sqrt(deg)
     so the whole tile is finished by one per-partition dinv scale.
Layer-1 epilogue also computes the layer-2 table tile (relu -> transpose ->
@W2 -> scale) so layer 2 only needs the second AllGather + aggregation.
"""
import sys
import os
import numpy as np
import ml_dtypes

sys.path.insert(0, "/opt/trn_rl_repo")

P = 128
H = 64
EDIM = 128
NCORES = 8
BLOCKS = 2          # src address blocks; signed int16 offsets from a base
                    # +32768 rows inside the block reach 65536 rows, so two
                    # blocks cover N=100000 with half the bucket-padding of 4
NSTR = 4            # gather streams (= SWDGE queues), 2 per block by parity
ABASE = 32768       # base row offset inside a block for signed idx addressing
GCHUNK = 8          # max chunks (of 128 idxs) per dma_gather instruction
                    # (>1024 idxs per instruction crashes the SWDGE ucode)
SBATCH = 8          # chunks per batched one-hot build
DSTW = 64           # dstloc columns per DMA load (multiple of SBATCH)

_COMPILED = {}


def _ceil_div(a, b):
    return (a + b - 1) // b


def make_schedule(edge_index, n_nodes, n_cores=NCORES):
    """Host-side marshaling: shard edges by dst owner, bucket by
    (dst_tile, src_block), pad each bucket to chunks of 128 with a shared
    chunk count across cores so all cores run an identical program."""
    src = edge_index[0].astype(np.int64)
    dst = edge_index[1].astype(np.int64)
    npc = n_nodes // n_cores
    T = _ceil_div(npc, P)
    BS = _ceil_div(n_nodes, BLOCKS)

    counts = np.zeros((n_cores, T, BLOCKS), np.int64)
    percore = []
    for c in range(n_cores):
        sel = (dst >= c * npc) & (dst < (c + 1) * npc)
        s = src[sel]
        d = dst[sel] - c * npc
        t = d >> 7
        b = s // BS
        key = t * BLOCKS + b
        order = np.argsort(key, kind="stable")
        s, d, key = s[order], d[order], key[order]
        cnt = np.bincount(key, minlength=T * BLOCKS).reshape(T, BLOCKS)
        counts[c] = cnt
        percore.append((s, d, cnt))

    K = _ceil_div(counts.max(axis=0), P).astype(np.int64)  # [T, BLOCKS]
    total_chunks = int(K.sum())

    # matmul-order index of chunk (t, b, k); chunk (t,b,k) is gathered by
    # stream 2b + ((k + t) & 1) so both queues of a block stay balanced
    m_start = np.zeros((T, BLOCKS), np.int64)
    pos_s = np.zeros((T, NSTR), np.int64)  # start of (t,b,par) run per stream
    S_s = np.zeros(NSTR, np.int64)
    m = 0
    for t in range(T):
        for b in range(BLOCKS):
            m_start[t, b] = m
            m += K[t, b]
            for par in range(2):
                s_ = 2 * b + par
                pos_s[t, s_] = S_s[s_]
                n_par = (int(K[t, b]) + (1 - ((par + t) & 1))) // 2
                S_s[s_] += n_par

    # per-stream gather instruction sizes (in chunks)
    gather_groups = []
    for s_ in range(NSTR):
        sizes = []
        rem = int(S_s[s_])
        while rem > 0:
            g = min(GCHUNK, rem)
            sizes.append(g)
            rem -= g
        gather_groups.append(sizes)

    # per-core data streams
    core_data = []
    for c in range(n_cores):
        s, d, cnt = percore[c]
        off = np.concatenate([[0], np.cumsum(cnt.reshape(-1))]).astype(np.int64)
        idx_streams = [np.zeros(max(int(S_s[s_]), 1) * P, np.int16)
                       for s_ in range(NSTR)]
        dstloc = np.full(total_chunks * P, -1.0, np.float32)
        for t in range(T):
            for b in range(BLOCKS):
                n = int(cnt[t, b])
                o = off[t * BLOCKS + b]
                ss = (s[o:o + n] - b * BS - ABASE).astype(np.int16)
                dd = (d[o:o + n] - t * P).astype(np.float32)
                for k in range(int(K[t, b])):
                    e0 = P * k
                    e1 = min(P * (k + 1), n)
                    if e1 <= e0:
                        break  # rest is padding: idx 0 = row ABASE, harmless
                    cs = ss[e0:e1].copy()
                    cdd = dd[e0:e1].copy()
                    if e1 - e0 == P and cs[-1] < 0:
                        # the SWDGE ucode trims trailing negative idxs, so a
                        # chunk may never end on one — swap a non-negative in
                        nz = np.nonzero(cs >= 0)[0]
                        assert len(nz), "all-negative gather chunk"
                        j = int(nz[0])
                        cs[j], cs[-1] = cs[-1], cs[j]
                        cdd[j], cdd[-1] = cdd[-1], cdd[j]
                    s_ = 2 * b + ((k + t) & 1)
                    q = (int(pos_s[t, s_]) + k // 2) * P
                    idx_streams[s_][q:q + (e1 - e0)] = cs
                    q0 = (int(m_start[t, b]) + k) * P
                    dstloc[q0:q0 + (e1 - e0)] = cdd
        # wrap idx streams for the gather ucode: [128, S_s*8] int16
        idx_wrapped = []
        for s_ in range(NSTR):
            w = idx_streams[s_].reshape(-1, 16).T          # [16, S_s*8]
            idx_wrapped.append(np.tile(w, (8, 1)).astype(np.int16))
        dst_t = dstloc.reshape(total_chunks, P).T.copy()  # [128, total_chunks]
        core_data.append((idx_wrapped, dst_t))

    return {
        "n_nodes": n_nodes, "n_cores": n_cores, "npc": npc, "T": T, "BS": BS,
        "K": K, "S_s": S_s.astype(np.int64), "total_chunks": total_chunks,
        "m_start": m_start, "pos_s": pos_s, "gather_groups": gather_groups,
        "core_data": core_data,
    }


def build_bass(sched, has_bias=True):
    from concourse import bass, bacc, tile, mybir

    n_cores = sched["n_cores"]
    npc = sched["npc"]
    T = sched["T"]
    N = sched["n_nodes"]
    BS = sched["BS"]
    K = sched["K"]
    S_s = sched["S_s"]
    total_chunks = sched["total_chunks"]
    m_start = sched["m_start"]
    pos_s = sched["pos_s"]
    gather_groups = sched["gather_groups"]
    f32 = mybir.dt.float32
    bf16 = mybir.dt.bfloat16
    i16 = mybir.dt.int16
    i32 = mybir.dt.int32

    nc = bacc.Bacc("TRN2", target_bir_lowering=False, debug=False,
                   enable_asserts=True, num_devices=n_cores,
                   num_swdge_queues=4)

    embT = nc.dram_tensor("embT", [P, T * P], f32, kind="ExternalInput")
    W1_d = nc.dram_tensor("W1", [EDIM, H], f32, kind="ExternalInput")
    W2_d = nc.dram_tensor("W2", [H, H], f32, kind="ExternalInput")
    b1_d = nc.dram_tensor("b1r", [1, H], f32, kind="ExternalInput")
    b2_d = nc.dram_tensor("b2r", [1, H], f32, kind="ExternalInput")
    dinv_d = nc.dram_tensor("dinv_t", [P, T], f32, kind="ExternalInput")
    sqd_d = nc.dram_tensor("sqd_row", [1, T * P], bf16, kind="ExternalInput")
    idx_d = [nc.dram_tensor(f"idx{s}", [P, max(int(S_s[s]), 1) * 8], i16,
                            kind="ExternalInput") for s in range(NSTR)]
    dst_d = nc.dram_tensor("dstloc", [P, max(total_chunks, 1)], f32,
                           kind="ExternalInput")
    out_d = nc.dram_tensor("out", [npc, H], f32, kind="ExternalOutput")

    with tile.TileContext(nc) as tc:
        with tc.tile_pool(name="const", bufs=1) as constp, \
             tc.tile_pool(name="tables", bufs=1) as tablep, \
             tc.tile_pool(name="work", bufs=3) as workp, \
             tc.tile_pool(name="idx", bufs=7) as idxp, \
             tc.tile_pool(name="gath", bufs=6) as gathp, \
             tc.tile_pool(name="gathb", bufs=8) as gathbp, \
             tc.tile_pool(name="spool", bufs=3) as spool, \
             tc.tile_pool(name="psum", bufs=4, space="PSUM") as psump, \
             tc.tile_pool(name="psumT", bufs=2, space="PSUM") as psumTp, \
             tc.tile_pool(name="dram", bufs=1, space="DRAM") as dramp:

            # ---- constants ----
            from concourse.masks import make_identity
            ident = constp.tile([P, P], bf16)
            make_identity(nc, ident[:])
            ident_f = constp.tile([P, P], f32)
            make_identity(nc, ident_f[:])
            iota_i = constp.tile([P, P], i32)
            nc.gpsimd.iota(iota_i[:], pattern=[[1, P]],
                           base=0, channel_multiplier=0)
            iota_f = constp.tile([P, P], bf16)
            nc.vector.tensor_copy(iota_f[:], iota_i[:])

            W1_s = constp.tile([EDIM, H], f32)
            nc.sync.dma_start(out=W1_s[:], in_=W1_d[:])
            W2_s = constp.tile([H, H], f32)
            nc.sync.dma_start(out=W2_s[:], in_=W2_d[:])
            b1_s = constp.tile([1, H], f32)
            nc.sync.dma_start(out=b1_s[:], in_=b1_d[:])
            b2_s = constp.tile([1, H], f32)
            nc.sync.dma_start(out=b2_s[:], in_=b2_d[:])
            dinv_s = constp.tile([P, T], f32)
            nc.sync.dma_start(out=dinv_s[:], in_=dinv_d[:])
            sqd_b = constp.tile([1, T * P], bf16)
            nc.sync.dma_start(out=sqd_b[:], in_=sqd_d[:])
            # bf16 copies for the scatter-path matmuls
            b1_b = constp.tile([1, H], bf16)
            nc.scalar.copy(b1_b[:], b1_s[:])
            b2_b = constp.tile([1, H], bf16)
            nc.scalar.copy(b2_b[:], b2_s[:])

            # ---- persistent tables in SBUF ----
            y1_all = tablep.tile([P, T * H], f32)     # layer-1 table, own slice
            y2_all = tablep.tile([P, T * H], f32)     # layer-2 table, own slice

            # ---- DRAM staging for collectives ----
            y1_in = dramp.tile([npc, H], f32)
            y2_in = dramp.tile([npc, H], f32)
            # Shared pair-HBM outputs let the AllGather skip per-core copies
            y1_full = nc.dram_tensor("y1_full_sh", [N, H], f32,
                                     addr_space="Shared")
            y2_full = nc.dram_tensor("y2_full_sh", [N, H], f32,
                                     addr_space="Shared")

            TH = (T * 3) // 4        # tiles in the early AllGather half
            # ---- phase 1: y1 = dinv * (emb @ W1) for own nodes ----
            ECH = 16                       # emb tiles per DMA batch
            for t0 in range(0, T, ECH):
                ntile = min(ECH, T - t0)
                xt = workp.tile([P, ECH * P], f32, tag="embT")
                nc.sync.dma_start(out=xt[:, :ntile * P],
                                  in_=embT[:, t0 * P:(t0 + ntile) * P])
                for dt_i in range(ntile):
                    t = t0 + dt_i
                    ps = psump.tile([P, H], f32, tag="ps")
                    nc.tensor.matmul(ps[:], lhsT=xt[:, dt_i * P:(dt_i + 1) * P],
                                     rhs=W1_s[:], start=True, stop=True)
                    ys = y1_all[:, t * H:(t + 1) * H]
                    nc.vector.tensor_scalar_mul(ys, ps[:], dinv_s[:, t:t + 1])
                    rows = min(npc - t * P, P)
                    nc.sync.dma_start(out=y1_in[t * P:t * P + rows, :],
                                      in_=y1_all[:rows, t * H:(t + 1) * H])

            # ---- phase 2: AllGather layer-1 table ----
            nc.gpsimd.collective_compute(
                "AllGather", mybir.AluOpType.bypass,
                replica_groups=[list(range(n_cores))],
                ins=[y1_in.opt()],
                outs=[y1_full[:, :].opt()],
            )

            # ---- aggregation pass (used for both layers) ----
            def aggregation(src_table, y_own, b_s, layer):
                # per-stream gather bookkeeping
                next_group = [0] * NSTR       # next gather group to issue
                group_start = [0] * NSTR      # chunk index where current group starts
                gbufs = [None] * NSTR
                sbuf_tile = [None]             # current one-hot batch tile
                sbatch_lo = [-1]
                dst_tile = [None]              # current dstloc load
                dstlo = [-1]

                prefix = [np.concatenate([[0], np.cumsum(gather_groups[s])])
                          .astype(int) for s in range(NSTR)]
                idx_fifo = [[] for _ in range(NSTR)]
                next_idx = [0] * NSTR
                IDX_AHEAD = 5

                def pump_idx(b):
                    # prefetch idx tiles ahead so gathers never stall the
                    # gpsimd sequencer waiting on a just-issued DMA. Only the
                    # 32 partitions queue b's ucode pair reads are loaded.
                    while (next_idx[b] < len(gather_groups[b])
                           and next_idx[b] < next_group[b] + IDX_AHEAD):
                        g = next_idx[b]
                        start = int(prefix[b][g])
                        size = gather_groups[b][g]
                        it = idxp.tile([P, GCHUNK * 8], i16, tag=f"idx{b}")
                        nc.sync.dma_start(
                            out=it[32 * b:32 * b + 32, :size * 8],
                            in_=idx_d[b][32 * b:32 * b + 32,
                                         start * 8:(start + size) * 8])
                        idx_fifo[b].append(it)
                        next_idx[b] += 1

                for s in range(NSTR):
                    pump_idx(s)

                def ensure_gather(b, pos):
                    while gbufs[b] is None or pos >= group_start[b] + gbufs[b][1]:
                        g = next_group[b]
                        start = int(prefix[b][g])
                        size = gather_groups[b][g]
                        it = idx_fifo[b].pop(0)
                        gt = gathp.tile([P, GCHUNK, H], f32, tag=f"g{b}")
                        base = (b // 2) * BS + ABASE
                        nc.gpsimd.dma_gather(
                            out_ap=gt[:, :size, :],
                            in_ap=src_table[base:min(base + ABASE, N), :],
                            idxs_ap=it[:, :size * 8],
                            num_idxs=size * P,
                            num_idxs_reg=size * P,
                            elem_size=H,
                            queue_num=b,
                        )
                        gtb = gathbp.tile([P, GCHUNK, H], bf16, tag=f"gb{b}")
                        nc.scalar.copy(gtb[:, :size, :], gt[:, :size, :])
                        gbufs[b] = (gtb, size)
                        group_start[b] = start
                        next_group[b] += 1
                        pump_idx(b)
                    return gbufs[b][0][:, pos - group_start[b], :]

                def ensure_s(m):
                    dlo = (m // DSTW) * DSTW
                    if dstlo != [dlo]:
                        dt_ = workp.tile([P, DSTW], f32, tag="dst")
                        w = min(DSTW, total_chunks - dlo)
                        nc.sync.dma_start(out=dt_[:, :w],
                                          in_=dst_d[:, dlo:dlo + w])
                        dst_tile[0] = dt_
                        dstlo[0] = dlo
                    st = spool.tile([P, P], bf16, tag="S")
                    nc.vector.tensor_scalar(
                        out=st[:], in0=iota_f[:],
                        scalar1=dst_tile[0][:, m - dstlo[0]:m - dstlo[0] + 1],
                        scalar2=None,
                        op0=mybir.AluOpType.is_equal)
                    return st[:]

                for t in range(T):
                    ps = psump.tile([P, H], f32, tag="ps")
                    first = True
                    for b in range(BLOCKS):
                        for k in range(int(K[t, b])):
                            s_ = 2 * b + ((k + t) & 1)
                            pos = int(pos_s[t, s_]) + k // 2
                            m = int(m_start[t, b]) + k
                            gview = ensure_gather(s_, pos)
                            sview = ensure_s(m)
                            nc.tensor.matmul(ps[:], lhsT=sview, rhs=gview,
                                             start=first, stop=False)
                            first = False
                    # self-loop: psum += y_own tile (cast to bf16 for the PE)
                    yb = workp.tile([P, H], bf16, tag="yb")
                    nc.scalar.copy(yb[:], y_own[:, t * H:(t + 1) * H])
                    nc.tensor.matmul(ps[:], lhsT=ident[:],
                                     rhs=yb[:],
                                     start=first, stop=not has_bias)
                    if has_bias:
                        # bias premultiplied by sqrt(deg): += sqd_j * b_d
                        nc.tensor.matmul(ps[:], lhsT=sqd_b[:, t * P:(t + 1) * P],
                                         rhs=b_s[:], start=False, stop=True)
                    yield t, ps

            # ---- phase 3: layer-1 aggregation + fused layer-2 table ----
            for t, ps in aggregation(y1_full, y1_all, b1_b, 1):
                h1 = workp.tile([P, H], f32, tag="h1")
                nc.scalar.activation(h1[:], ps[:],
                                     mybir.ActivationFunctionType.Relu,
                                     scale=dinv_s[:, t:t + 1])
                pT = psumTp.tile([H, P], f32)
                nc.tensor.transpose(pT[:], h1[:], ident_f[:])
                h1T = workp.tile([H, P], f32, tag="h1T")
                nc.vector.tensor_copy(h1T[:], pT[:])
                ps2 = psump.tile([P, H], f32, tag="ps")
                nc.tensor.matmul(ps2[:], lhsT=h1T[:], rhs=W2_s[:],
                                 start=True, stop=True)
                y2s = y2_all[:, t * H:(t + 1) * H]
                nc.vector.tensor_scalar_mul(y2s, ps2[:], dinv_s[:, t:t + 1])
                rows = min(npc - t * P, P)
                nc.sync.dma_start(out=y2_in[t * P:t * P + rows, :],
                                  in_=y2_all[:rows, t * H:(t + 1) * H])

            # ---- phase 4: AllGather layer-2 table ----
            nc.gpsimd.collective_compute(
                "AllGather", mybir.AluOpType.bypass,
                replica_groups=[list(range(n_cores))],
                ins=[y2_in.opt()],
                outs=[y2_full[:, :].opt()],
            )

            # ---- phase 5: layer-2 aggregation -> output ----
            for t, ps in aggregation(y2_full, y2_all, b2_b, 2):
                ot = workp.tile([P, H], f32, tag="ot")
                nc.vector.tensor_scalar_mul(ot[:], ps[:], dinv_s[:, t:t + 1])
                rows = min(npc - t * P, P)
                nc.sync.dma_start(out=out_d[t * P:t * P + rows, :],
                                  in_=ot[:rows, :])

    nc.compile()
    return nc


def make_inputs(sched, emb_weight, W1, b1, W2, b2, deg):
    """Build per-core input maps."""
    n_cores = sched["n_cores"]
    npc = sched["npc"]
    T = sched["T"]
    dinv = (1.0 / np.sqrt(deg.astype(np.float64))).astype(np.float32)
    sqd = np.sqrt(deg.astype(np.float64)).astype(np.float32)
    in_maps = []
    for c in range(n_cores):
        lo, hi = c * npc, (c + 1) * npc
        embT = np.zeros((P, T * P), np.float32)
        embT[:, :npc] = emb_weight[lo:hi].T
        tmp = np.zeros(T * P, np.float32)
        tmp[:npc] = dinv[lo:hi]
        dinv_t = np.ascontiguousarray(tmp.reshape(T, P).T)
        sqd_row = np.zeros((1, T * P), np.float32)
        sqd_row[0, :npc] = sqd[lo:hi]
        idx_wrapped, dst_t = sched["core_data"][c]
        m = {
            "embT": embT,
            "W1": W1.astype(np.float32),
            "W2": W2.astype(np.float32),
            "b1r": b1.reshape(1, -1).astype(np.float32),
            "b2r": b2.reshape(1, -1).astype(np.float32),
            "dinv_t": dinv_t,
            "sqd_row": sqd_row.astype(ml_dtypes.bfloat16),
            "dstloc": dst_t,
        }
        for s in range(NSTR):
            iw = idx_wrapped[s]
            if iw.shape[1] == 0:
                iw = np.zeros((P, 8), np.int16)
            m[f"idx{s}"] = iw
        in_maps.append(m)
    return in_maps


def run(edge_index, emb_weight, W1, b1, W2, b2, n_nodes=None, trace=False):
    from concourse import bass_utils
    n_nodes = n_nodes if n_nodes is not None else emb_weight.shape[0]
    sched = make_schedule(np.asarray(edge_index), n_nodes)
    has_bias = bool(np.any(np.asarray(b1)) or np.any(np.asarray(b2)))
    key = ("gnn", n_nodes, int(sched["total_chunks"]), has_bias,
           tuple(int(x) for x in sched["S_s"]))
    if key not in _COMPILED:
        _COMPILED[key] = build_bass(sched, has_bias)
    nc = _COMPILED[key]
    deg = np.bincount(np.asarray(edge_index)[1], minlength=n_nodes).astype(np.float32) + 1.0
    in_maps = make_inputs(sched, np.asarray(emb_weight), np.asarray(W1),
                          np.asarray(b1), np.asarray(W2), np.asarray(b2), deg)
    res = bass_utils.run_bass_kernel_spmd(
        nc, in_maps, core_ids=list(range(sched["n_cores"])), trace=trace)
    npc = sched["npc"]
    out = np.concatenate([res.results[c]["out"] for c in range(sched["n_cores"])],
                         axis=0)
    return out[:n_nodes], res


def kernel(edge_index, emb_weight, W1, b1, W2, b2):
    out, _ = run(edge_index, emb_weight, W1, b1, W2, b2)
    return out



# revision 34
# speedup vs baseline: 1.4803x; 1.4803x over previous
"""Trainium2 Bass kernel for a 2-layer GCN (NextHopGNN).

Distribution: destination-node sharding across 8 NeuronCores. Each core owns
N/8 destination nodes and all edges pointing into them. Per layer:
  1. each core computes its slice of the scaled feature table
     y = dinv * (x @ W)  (PE matmuls, dinv = 1/sqrt(degree) with self-loops)
  2. AllGather -> every core holds the full [N, 64] table in HBM
  3. per 128-node dst tile: dma_gather the edge source rows (256B rows,
     int16 indices relative to one of 4 source blocks), build one-hot
     scatter matrices on the vector engine (batched iota == dstloc), and
     accumulate S^T @ G in PSUM on the tensor engine. Self-loop added via an
     identity matmul, bias via a rank-1 matmul pre-multiplied by sqrt(deg)
     so the whole tile is finished by one per-partition dinv scale.
Layer-1 epilogue also computes the layer-2 table tile (relu -> transpose ->
@W2 -> scale) so layer 2 only needs the second AllGather + aggregation.
"""
import sys
import os
import numpy as np
import ml_dtypes

sys.path.insert(0, "/opt/trn_rl_repo")

P = 128
H = 64
EDIM = 128
NCORES = 8
BLOCKS = 2          # src address blocks; signed int16 offsets from a base
                    # +32768 rows inside the block reach 65536 rows, so two
                    # blocks cover N=100000 with half the bucket-padding of 4
NSTR = 4            # gather streams (= SWDGE queues), 2 per block by parity
ABASE = 32768       # base row offset inside a block for signed idx addressing
GCHUNK = 8          # max chunks (of 128 idxs) per dma_gather instruction
                    # (>1024 idxs per instruction crashes the SWDGE ucode)
SBATCH = 8          # chunks per batched one-hot build
DSTW = 64           # dstloc columns per DMA load (multiple of SBATCH)

_COMPILED = {}


def _ceil_div(a, b):
    return (a + b - 1) // b


def make_schedule(edge_index, n_nodes, n_cores=NCORES):
    """Host-side marshaling: shard edges by dst owner, bucket by
    (dst_tile, src_block), pad each bucket to chunks of 128 with a shared
    chunk count across cores so all cores run an identical program."""
    src = edge_index[0].astype(np.int64)
    dst = edge_index[1].astype(np.int64)
    npc = n_nodes // n_cores
    T = _ceil_div(npc, P)
    BS = _ceil_div(n_nodes, BLOCKS)

    counts = np.zeros((n_cores, T, BLOCKS), np.int64)
    percore = []
    for c in range(n_cores):
        sel = (dst >= c * npc) & (dst < (c + 1) * npc)
        s = src[sel]
        d = dst[sel] - c * npc
        t = d >> 7
        b = s // BS
        key = t * BLOCKS + b
        order = np.argsort(key, kind="stable")
        s, d, key = s[order], d[order], key[order]
        cnt = np.bincount(key, minlength=T * BLOCKS).reshape(T, BLOCKS)
        counts[c] = cnt
        percore.append((s, d, cnt))

    K = _ceil_div(counts.max(axis=0), P).astype(np.int64)  # [T, BLOCKS]
    total_chunks = int(K.sum())

    # matmul-order index of chunk (t, b, k); chunk (t,b,k) is gathered by
    # stream 2b + ((k + t) & 1) so both queues of a block stay balanced
    m_start = np.zeros((T, BLOCKS), np.int64)
    pos_s = np.zeros((T, NSTR), np.int64)  # start of (t,b,par) run per stream
    S_s = np.zeros(NSTR, np.int64)
    m = 0
    for t in range(T):
        for b in range(BLOCKS):
            m_start[t, b] = m
            m += K[t, b]
            for par in range(2):
                s_ = 2 * b + par
                pos_s[t, s_] = S_s[s_]
                n_par = (int(K[t, b]) + (1 - ((par + t) & 1))) // 2
                S_s[s_] += n_par

    # per-stream gather instruction sizes (in chunks)
    gather_groups = []
    for s_ in range(NSTR):
        sizes = []
        rem = int(S_s[s_])
        while rem > 0:
            g = min(GCHUNK, rem)
            sizes.append(g)
            rem -= g
        gather_groups.append(sizes)

    # per-core data streams
    core_data = []
    for c in range(n_cores):
        s, d, cnt = percore[c]
        off = np.concatenate([[0], np.cumsum(cnt.reshape(-1))]).astype(np.int64)
        idx_streams = [np.zeros(max(int(S_s[s_]), 1) * P, np.int16)
                       for s_ in range(NSTR)]
        dstloc = np.full(total_chunks * P, -1.0, np.float32)
        for t in range(T):
            for b in range(BLOCKS):
                n = int(cnt[t, b])
                o = off[t * BLOCKS + b]
                ss = (s[o:o + n] - b * BS - ABASE).astype(np.int16)
                dd = (d[o:o + n] - t * P).astype(np.float32)
                for k in range(int(K[t, b])):
                    e0 = P * k
                    e1 = min(P * (k + 1), n)
                    if e1 <= e0:
                        break  # rest is padding: idx 0 = row ABASE, harmless
                    cs = ss[e0:e1].copy()
                    cdd = dd[e0:e1].copy()
                    if e1 - e0 == P and cs[-1] < 0:
                        # the SWDGE ucode trims trailing negative idxs, so a
                        # chunk may never end on one — swap a non-negative in
                        nz = np.nonzero(cs >= 0)[0]
                        assert len(nz), "all-negative gather chunk"
                        j = int(nz[0])
                        cs[j], cs[-1] = cs[-1], cs[j]
                        cdd[j], cdd[-1] = cdd[-1], cdd[j]
                    s_ = 2 * b + ((k + t) & 1)
                    q = (int(pos_s[t, s_]) + k // 2) * P
                    idx_streams[s_][q:q + (e1 - e0)] = cs
                    q0 = (int(m_start[t, b]) + k) * P
                    dstloc[q0:q0 + (e1 - e0)] = cdd
        # wrap idx streams for the gather ucode: [128, S_s*8] int16
        idx_wrapped = []
        for s_ in range(NSTR):
            w = idx_streams[s_].reshape(-1, 16).T          # [16, S_s*8]
            idx_wrapped.append(np.tile(w, (8, 1)).astype(np.int16))
        dst_t = dstloc.reshape(total_chunks, P).T.copy()  # [128, total_chunks]
        core_data.append((idx_wrapped, dst_t))

    return {
        "n_nodes": n_nodes, "n_cores": n_cores, "npc": npc, "T": T, "BS": BS,
        "K": K, "S_s": S_s.astype(np.int64), "total_chunks": total_chunks,
        "m_start": m_start, "pos_s": pos_s, "gather_groups": gather_groups,
        "core_data": core_data,
    }


def build_bass(sched, has_bias=True):
    from concourse import bass, bacc, tile, mybir

    n_cores = sched["n_cores"]
    npc = sched["npc"]
    T = sched["T"]
    N = sched["n_nodes"]
    BS = sched["BS"]
    K = sched["K"]
    S_s = sched["S_s"]
    total_chunks = sched["total_chunks"]
    m_start = sched["m_start"]
    pos_s = sched["pos_s"]
    gather_groups = sched["gather_groups"]
    f32 = mybir.dt.float32
    bf16 = mybir.dt.bfloat16
    i16 = mybir.dt.int16
    i32 = mybir.dt.int32

    nc = bacc.Bacc("TRN2", target_bir_lowering=False, debug=False,
                   enable_asserts=True, num_devices=n_cores,
                   num_swdge_queues=4)

    embT = nc.dram_tensor("embT", [P, T * P], f32, kind="ExternalInput")
    W1_d = nc.dram_tensor("W1", [EDIM, H], f32, kind="ExternalInput")
    W2_d = nc.dram_tensor("W2", [H, H], f32, kind="ExternalInput")
    b1_d = nc.dram_tensor("b1r", [1, H], f32, kind="ExternalInput")
    b2_d = nc.dram_tensor("b2r", [1, H], f32, kind="ExternalInput")
    dinv_d = nc.dram_tensor("dinv_t", [P, T], f32, kind="ExternalInput")
    sqd_d = nc.dram_tensor("sqd_row", [1, T * P], bf16, kind="ExternalInput")
    idx_d = [nc.dram_tensor(f"idx{s}", [P, max(int(S_s[s]), 1) * 8], i16,
                            kind="ExternalInput") for s in range(NSTR)]
    dst_d = nc.dram_tensor("dstloc", [P, max(total_chunks, 1)], bf16,
                           kind="ExternalInput")
    out_d = nc.dram_tensor("out", [npc, H], f32, kind="ExternalOutput")

    with tile.TileContext(nc) as tc:
        with tc.tile_pool(name="const", bufs=1) as constp, \
             tc.tile_pool(name="tables", bufs=1) as tablep, \
             tc.tile_pool(name="work", bufs=3) as workp, \
             tc.tile_pool(name="idx", bufs=7) as idxp, \
             tc.tile_pool(name="gath", bufs=6) as gathp, \
             tc.tile_pool(name="gathb", bufs=8) as gathbp, \
             tc.tile_pool(name="spool", bufs=3) as spool, \
             tc.tile_pool(name="psum", bufs=4, space="PSUM") as psump, \
             tc.tile_pool(name="psumT", bufs=2, space="PSUM") as psumTp, \
             tc.tile_pool(name="dram", bufs=1, space="DRAM") as dramp:

            # ---- constants ----
            from concourse.masks import make_identity
            ident = constp.tile([P, P], bf16)
            make_identity(nc, ident[:])
            ident_f = constp.tile([P, P], f32)
            make_identity(nc, ident_f[:])
            iota_i = constp.tile([P, SBATCH * P], i32)
            nc.gpsimd.iota(iota_i[:], pattern=[[0, SBATCH], [1, P]],
                           base=0, channel_multiplier=0)
            iota_f = constp.tile([P, SBATCH * P], bf16)
            nc.vector.tensor_copy(iota_f[:], iota_i[:])

            W1_s = constp.tile([EDIM, H], f32)
            nc.sync.dma_start(out=W1_s[:], in_=W1_d[:])
            W2_s = constp.tile([H, H], f32)
            nc.sync.dma_start(out=W2_s[:], in_=W2_d[:])
            b1_s = constp.tile([1, H], f32)
            nc.sync.dma_start(out=b1_s[:], in_=b1_d[:])
            b2_s = constp.tile([1, H], f32)
            nc.sync.dma_start(out=b2_s[:], in_=b2_d[:])
            dinv_s = constp.tile([P, T], f32)
            nc.sync.dma_start(out=dinv_s[:], in_=dinv_d[:])
            sqd_b = constp.tile([1, T * P], bf16)
            nc.sync.dma_start(out=sqd_b[:], in_=sqd_d[:])
            # bf16 copies for the scatter-path matmuls
            b1_b = constp.tile([1, H], bf16)
            nc.scalar.copy(b1_b[:], b1_s[:])
            b2_b = constp.tile([1, H], bf16)
            nc.scalar.copy(b2_b[:], b2_s[:])

            # ---- persistent tables in SBUF ----
            y1_all = tablep.tile([P, T * H], f32)     # layer-1 table, own slice
            y2_all = tablep.tile([P, T * H], f32)     # layer-2 table, own slice

            # ---- DRAM staging for collectives ----
            y1_in = dramp.tile([npc, H], f32)
            y2_in = dramp.tile([npc, H], f32)
            # Shared pair-HBM outputs let the AllGather skip per-core copies
            y1_full = nc.dram_tensor("y1_full_sh", [N, H], f32,
                                     addr_space="Shared")
            y2_full = nc.dram_tensor("y2_full_sh", [N, H], f32,
                                     addr_space="Shared")

            TH = (T * 3) // 4        # tiles in the early AllGather half
            # ---- phase 1: y1 = dinv * (emb @ W1) for own nodes ----
            ECH = 16                       # emb tiles per DMA batch
            for t0 in range(0, T, ECH):
                ntile = min(ECH, T - t0)
                xt = workp.tile([P, ECH * P], f32, tag="embT")
                nc.sync.dma_start(out=xt[:, :ntile * P],
                                  in_=embT[:, t0 * P:(t0 + ntile) * P])
                for dt_i in range(ntile):
                    t = t0 + dt_i
                    ps = psump.tile([P, H], f32, tag="ps")
                    nc.tensor.matmul(ps[:], lhsT=xt[:, dt_i * P:(dt_i + 1) * P],
                                     rhs=W1_s[:], start=True, stop=True)
                    ys = y1_all[:, t * H:(t + 1) * H]
                    nc.vector.tensor_scalar_mul(ys, ps[:], dinv_s[:, t:t + 1])
                    rows = min(npc - t * P, P)
                    nc.sync.dma_start(out=y1_in[t * P:t * P + rows, :],
                                      in_=y1_all[:rows, t * H:(t + 1) * H])

            # ---- phase 2: AllGather layer-1 table ----
            nc.gpsimd.collective_compute(
                "AllGather", mybir.AluOpType.bypass,
                replica_groups=[list(range(n_cores))],
                ins=[y1_in.opt()],
                outs=[y1_full[:, :].opt()],
            )

            # ---- aggregation pass (used for both layers) ----
            def aggregation(src_table, y_own, b_s, layer):
                # per-stream gather bookkeeping
                next_group = [0] * NSTR       # next gather group to issue
                group_start = [0] * NSTR      # chunk index where current group starts
                gbufs = [None] * NSTR
                sbuf_tile = [None]             # current one-hot batch tile
                sbatch_lo = [-1]
                dst_tile = [None]              # current dstloc load
                dstlo = [-1]

                prefix = [np.concatenate([[0], np.cumsum(gather_groups[s])])
                          .astype(int) for s in range(NSTR)]
                idx_fifo = [[] for _ in range(NSTR)]
                next_idx = [0] * NSTR
                IDX_AHEAD = 5

                def pump_idx(b):
                    # prefetch idx tiles ahead so gathers never stall the
                    # gpsimd sequencer waiting on a just-issued DMA. Only the
                    # 32 partitions queue b's ucode pair reads are loaded.
                    while (next_idx[b] < len(gather_groups[b])
                           and next_idx[b] < next_group[b] + IDX_AHEAD):
                        g = next_idx[b]
                        start = int(prefix[b][g])
                        size = gather_groups[b][g]
                        it = idxp.tile([P, GCHUNK * 8], i16, tag=f"idx{b}")
                        nc.sync.dma_start(
                            out=it[32 * b:32 * b + 32, :size * 8],
                            in_=idx_d[b][32 * b:32 * b + 32,
                                         start * 8:(start + size) * 8])
                        idx_fifo[b].append(it)
                        next_idx[b] += 1

                for s in range(NSTR):
                    pump_idx(s)

                def ensure_gather(b, pos):
                    while gbufs[b] is None or pos >= group_start[b] + gbufs[b][1]:
                        g = next_group[b]
                        start = int(prefix[b][g])
                        size = gather_groups[b][g]
                        it = idx_fifo[b].pop(0)
                        gt = gathp.tile([P, GCHUNK, H], f32, tag=f"g{b}")
                        base = (b // 2) * BS + ABASE
                        nc.gpsimd.dma_gather(
                            out_ap=gt[:, :size, :],
                            in_ap=src_table[base:min(base + ABASE, N), :],
                            idxs_ap=it[:, :size * 8],
                            num_idxs=size * P,
                            num_idxs_reg=size * P,
                            elem_size=H,
                            queue_num=b,
                        )
                        gtb = gathbp.tile([P, GCHUNK, H], bf16, tag=f"gb{b}")
                        nc.scalar.copy(gtb[:, :size, :], gt[:, :size, :])
                        gbufs[b] = (gtb, size)
                        group_start[b] = start
                        next_group[b] += 1
                        pump_idx(b)
                    return gbufs[b][0][:, pos - group_start[b], :]

                def ensure_s(m):
                    lo = (m // SBATCH) * SBATCH
                    if sbatch_lo[0] != lo:
                        dlo = (m // DSTW) * DSTW
                        if dstlo != [dlo]:
                            dt_ = workp.tile([P, DSTW], bf16, tag="dst")
                            w = min(DSTW, total_chunks - dlo)
                            nc.sync.dma_start(out=dt_[:, :w],
                                              in_=dst_d[:, dlo:dlo + w])
                            dst_tile[0] = dt_
                            dstlo[0] = dlo
                        nb = min(SBATCH, total_chunks - lo)
                        st = spool.tile([P, SBATCH * P], bf16, tag="S")
                        col = lo - dstlo[0]
                        dl = dst_tile[0][:, col:col + nb]
                        dl3 = dl.rearrange("p (c u) -> p c u", u=1)
                        nc.vector.tensor_tensor(
                            out=st[:, :nb * P].rearrange("p (c j) -> p c j", j=P),
                            in0=iota_f[:, :nb * P].rearrange("p (c j) -> p c j", j=P),
                            in1=dl3.to_broadcast([P, nb, P]),
                            op=mybir.AluOpType.is_equal)
                        sbuf_tile[0] = st
                        sbatch_lo[0] = lo
                    return sbuf_tile[0][:, (m - sbatch_lo[0]) * P:
                                        (m - sbatch_lo[0] + 1) * P]

                for t in range(T):
                    ps = psump.tile([P, H], f32, tag="ps")
                    first = True
                    for b in range(BLOCKS):
                        for k in range(int(K[t, b])):
                            s_ = 2 * b + ((k + t) & 1)
                            pos = int(pos_s[t, s_]) + k // 2
                            m = int(m_start[t, b]) + k
                            gview = ensure_gather(s_, pos)
                            sview = ensure_s(m)
                            nc.tensor.matmul(ps[:], lhsT=sview, rhs=gview,
                                             start=first, stop=False)
                            first = False
                    # self-loop: psum += y_own tile (cast to bf16 for the PE)
                    yb = workp.tile([P, H], bf16, tag="yb")
                    nc.scalar.copy(yb[:], y_own[:, t * H:(t + 1) * H])
                    nc.tensor.matmul(ps[:], lhsT=ident[:],
                                     rhs=yb[:],
                                     start=first, stop=not has_bias)
                    if has_bias:
                        # bias premultiplied by sqrt(deg): += sqd_j * b_d
                        nc.tensor.matmul(ps[:], lhsT=sqd_b[:, t * P:(t + 1) * P],
                                         rhs=b_s[:], start=False, stop=True)
                    yield t, ps

            # ---- phase 3: layer-1 aggregation + fused layer-2 table ----
            for t, ps in aggregation(y1_full, y1_all, b1_b, 1):
                h1 = workp.tile([P, H], f32, tag="h1")
                nc.scalar.activation(h1[:], ps[:],
                                     mybir.ActivationFunctionType.Relu,
                                     scale=dinv_s[:, t:t + 1])
                pT = psumTp.tile([H, P], f32)
                nc.tensor.transpose(pT[:], h1[:], ident_f[:])
                h1T = workp.tile([H, P], f32, tag="h1T")
                nc.vector.tensor_copy(h1T[:], pT[:])
                ps2 = psump.tile([P, H], f32, tag="ps")
                nc.tensor.matmul(ps2[:], lhsT=h1T[:], rhs=W2_s[:],
                                 start=True, stop=True)
                y2s = y2_all[:, t * H:(t + 1) * H]
                nc.vector.tensor_scalar_mul(y2s, ps2[:], dinv_s[:, t:t + 1])
                rows = min(npc - t * P, P)
                nc.sync.dma_start(out=y2_in[t * P:t * P + rows, :],
                                  in_=y2_all[:rows, t * H:(t + 1) * H])

            # ---- phase 4: AllGather layer-2 table ----
            nc.gpsimd.collective_compute(
                "AllGather", mybir.AluOpType.bypass,
                replica_groups=[list(range(n_cores))],
                ins=[y2_in.opt()],
                outs=[y2_full[:, :].opt()],
            )

            # ---- phase 5: layer-2 aggregation -> output ----
            for t, ps in aggregation(y2_full, y2_all, b2_b, 2):
                ot = workp.tile([P, H], f32, tag="ot")
                nc.vector.tensor_scalar_mul(ot[:], ps[:], dinv_s[:, t:t + 1])
                rows = min(npc - t * P, P)
                nc.sync.dma_start(out=out_d[t * P:t * P + rows, :],
                                  in_=ot[:rows, :])

    nc.compile()
    return nc


def make_inputs(sched, emb_weight, W1, b1, W2, b2, deg):
    """Build per-core input maps."""
    n_cores = sched["n_cores"]
    npc = sched["npc"]
    T = sched["T"]
    dinv = (1.0 / np.sqrt(deg.astype(np.float64))).astype(np.float32)
    sqd = np.sqrt(deg.astype(np.float64)).astype(np.float32)
    in_maps = []
    for c in range(n_cores):
        lo, hi = c * npc, (c + 1) * npc
        embT = np.zeros((P, T * P), np.float32)
        embT[:, :npc] = emb_weight[lo:hi].T
        tmp = np.zeros(T * P, np.float32)
        tmp[:npc] = dinv[lo:hi]
        dinv_t = np.ascontiguousarray(tmp.reshape(T, P).T)
        sqd_row = np.zeros((1, T * P), np.float32)
        sqd_row[0, :npc] = sqd[lo:hi]
        idx_wrapped, dst_t = sched["core_data"][c]
        m = {
            "embT": embT,
            "W1": W1.astype(np.float32),
            "W2": W2.astype(np.float32),
            "b1r": b1.reshape(1, -1).astype(np.float32),
            "b2r": b2.reshape(1, -1).astype(np.float32),
            "dinv_t": dinv_t,
            "sqd_row": sqd_row.astype(ml_dtypes.bfloat16),
            "dstloc": dst_t.astype(ml_dtypes.bfloat16),
        }
        for s in range(NSTR):
            iw = idx_wrapped[s]
            if iw.shape[1] == 0:
                iw = np.zeros((P, 8), np.int16)
            m[f"idx{s}"] = iw
        in_maps.append(m)
    return in_maps


def run(edge_index, emb_weight, W1, b1, W2, b2, n_nodes=None, trace=False):
    from concourse import bass_utils
    n_nodes = n_nodes if n_nodes is not None else emb_weight.shape[0]
    sched = make_schedule(np.asarray(edge_index), n_nodes)
    has_bias = bool(np.any(np.asarray(b1)) or np.any(np.asarray(b2)))
    key = ("gnn", n_nodes, int(sched["total_chunks"]), has_bias,
           tuple(int(x) for x in sched["S_s"]))
    if key not in _COMPILED:
        _COMPILED[key] = build_bass(sched, has_bias)
    nc = _COMPILED[key]
    deg = np.bincount(np.asarray(edge_index)[1], minlength=n_nodes).astype(np.float32) + 1.0
    in_maps = make_inputs(sched, np.asarray(emb_weight), np.asarray(W1),
                          np.asarray(b1), np.asarray(W2), np.asarray(b2), deg)
    res = bass_utils.run_bass_kernel_spmd(
        nc, in_maps, core_ids=list(range(sched["n_cores"])), trace=trace)
    npc = sched["npc"]
    out = np.concatenate([res.results[c]["out"] for c in range(sched["n_cores"])],
                         axis=0)
    return out[:n_nodes], res


def kernel(edge_index, emb_weight, W1, b1, W2, b2):
    out, _ = run(edge_index, emb_weight, W1, b1, W2, b2)
    return out



# revision 35
# speedup vs baseline: 1.5943x; 1.0770x over previous
"""Trainium2 Bass kernel for a 2-layer GCN (NextHopGNN).

Distribution: destination-node sharding across 8 NeuronCores. Each core owns
N/8 destination nodes and all edges pointing into them. Per layer:
  1. each core computes its slice of the scaled feature table
     y = dinv * (x @ W)  (PE matmuls, dinv = 1/sqrt(degree) with self-loops)
  2. AllGather -> every core holds the full [N, 64] table in HBM
  3. per 128-node dst tile: dma_gather the edge source rows (256B rows,
     int16 indices relative to one of 4 source blocks), build one-hot
     scatter matrices on the vector engine (batched iota == dstloc), and
     accumulate S^T @ G in PSUM on the tensor engine. Self-loop added via an
     identity matmul, bias via a rank-1 matmul pre-multiplied by sqrt(deg)
     so the whole tile is finished by one per-partition dinv scale.
Layer-1 epilogue also computes the layer-2 table tile (relu -> transpose ->
@W2 -> scale) so layer 2 only needs the second AllGather + aggregation.
"""
import sys
import os
import numpy as np
import ml_dtypes

sys.path.insert(0, "/opt/trn_rl_repo")

P = 128
H = 64
EDIM = 128
NCORES = 8
BLOCKS = 2          # src address blocks; signed int16 offsets from a base
                    # +32768 rows inside the block reach 65536 rows, so two
                    # blocks cover N=100000 with half the bucket-padding of 4
NSTR = 4            # gather streams (= SWDGE queues), 2 per block by parity
ABASE = 32768       # base row offset inside a block for signed idx addressing
GCHUNK = 8          # max chunks (of 128 idxs) per dma_gather instruction
                    # (>1024 idxs per instruction crashes the SWDGE ucode)
SBATCH = 8          # chunks per batched one-hot build
DSTW = 64           # dstloc columns per DMA load (multiple of SBATCH)

_COMPILED = {}


def _ceil_div(a, b):
    return (a + b - 1) // b


def make_schedule(edge_index, n_nodes, n_cores=NCORES):
    """Host-side marshaling: shard edges by dst owner, bucket by
    (dst_tile, src_block), pad each bucket to chunks of 128 with a shared
    chunk count across cores so all cores run an identical program."""
    src = edge_index[0].astype(np.int64)
    dst = edge_index[1].astype(np.int64)
    npc = n_nodes // n_cores
    T = _ceil_div(npc, P)
    BS = _ceil_div(n_nodes, BLOCKS)

    counts = np.zeros((n_cores, T, BLOCKS), np.int64)
    percore = []
    for c in range(n_cores):
        sel = (dst >= c * npc) & (dst < (c + 1) * npc)
        s = src[sel]
        d = dst[sel] - c * npc
        t = d >> 7
        b = s // BS
        key = t * BLOCKS + b
        order = np.argsort(key, kind="stable")
        s, d, key = s[order], d[order], key[order]
        cnt = np.bincount(key, minlength=T * BLOCKS).reshape(T, BLOCKS)
        counts[c] = cnt
        percore.append((s, d, cnt))

    K = _ceil_div(counts.max(axis=0), P).astype(np.int64)  # [T, BLOCKS]
    total_chunks = int(K.sum())

    # matmul-order index of chunk (t, b, k); chunk (t,b,k) is gathered by
    # stream 2b + ((k + t) & 1) so both queues of a block stay balanced
    m_start = np.zeros((T, BLOCKS), np.int64)
    pos_s = np.zeros((T, NSTR), np.int64)  # start of (t,b,par) run per stream
    S_s = np.zeros(NSTR, np.int64)
    m = 0
    for t in range(T):
        for b in range(BLOCKS):
            m_start[t, b] = m
            m += K[t, b]
            for par in range(2):
                s_ = 2 * b + par
                pos_s[t, s_] = S_s[s_]
                n_par = (int(K[t, b]) + (1 - ((par + t) & 1))) // 2
                S_s[s_] += n_par

    # per-stream gather instruction sizes (in chunks)
    gather_groups = []
    for s_ in range(NSTR):
        sizes = []
        rem = int(S_s[s_])
        while rem > 0:
            g = min(GCHUNK, rem)
            sizes.append(g)
            rem -= g
        gather_groups.append(sizes)

    # per-core data streams
    core_data = []
    for c in range(n_cores):
        s, d, cnt = percore[c]
        off = np.concatenate([[0], np.cumsum(cnt.reshape(-1))]).astype(np.int64)
        idx_streams = [np.zeros(max(int(S_s[s_]), 1) * P, np.int16)
                       for s_ in range(NSTR)]
        dstloc = np.full(total_chunks * P, -1.0, np.float32)
        for t in range(T):
            for b in range(BLOCKS):
                n = int(cnt[t, b])
                o = off[t * BLOCKS + b]
                ss = (s[o:o + n] - b * BS - ABASE).astype(np.int16)
                dd = (d[o:o + n] - t * P).astype(np.float32)
                for k in range(int(K[t, b])):
                    e0 = P * k
                    e1 = min(P * (k + 1), n)
                    if e1 <= e0:
                        break  # rest is padding: idx 0 = row ABASE, harmless
                    cs = ss[e0:e1].copy()
                    cdd = dd[e0:e1].copy()
                    if e1 - e0 == P and cs[-1] < 0:
                        # the SWDGE ucode trims trailing negative idxs, so a
                        # chunk may never end on one — swap a non-negative in
                        nz = np.nonzero(cs >= 0)[0]
                        assert len(nz), "all-negative gather chunk"
                        j = int(nz[0])
                        cs[j], cs[-1] = cs[-1], cs[j]
                        cdd[j], cdd[-1] = cdd[-1], cdd[j]
                    s_ = 2 * b + ((k + t) & 1)
                    q = (int(pos_s[t, s_]) + k // 2) * P
                    idx_streams[s_][q:q + (e1 - e0)] = cs
                    q0 = (int(m_start[t, b]) + k) * P
                    dstloc[q0:q0 + (e1 - e0)] = cdd
        # wrap idx streams for the gather ucode: [128, S_s*8] int16
        idx_wrapped = []
        for s_ in range(NSTR):
            w = idx_streams[s_].reshape(-1, 16).T          # [16, S_s*8]
            idx_wrapped.append(np.tile(w, (8, 1)).astype(np.int16))
        dst_t = dstloc.reshape(total_chunks, P).T.copy()  # [128, total_chunks]
        core_data.append((idx_wrapped, dst_t))

    return {
        "n_nodes": n_nodes, "n_cores": n_cores, "npc": npc, "T": T, "BS": BS,
        "K": K, "S_s": S_s.astype(np.int64), "total_chunks": total_chunks,
        "m_start": m_start, "pos_s": pos_s, "gather_groups": gather_groups,
        "core_data": core_data,
    }


def build_bass(sched, has_bias=True):
    from concourse import bass, bacc, tile, mybir

    n_cores = sched["n_cores"]
    npc = sched["npc"]
    T = sched["T"]
    N = sched["n_nodes"]
    BS = sched["BS"]
    K = sched["K"]
    S_s = sched["S_s"]
    total_chunks = sched["total_chunks"]
    m_start = sched["m_start"]
    pos_s = sched["pos_s"]
    gather_groups = sched["gather_groups"]
    f32 = mybir.dt.float32
    bf16 = mybir.dt.bfloat16
    i16 = mybir.dt.int16
    i32 = mybir.dt.int32

    nc = bacc.Bacc("TRN2", target_bir_lowering=False, debug=False,
                   enable_asserts=True, num_devices=n_cores,
                   num_swdge_queues=4)

    embT = nc.dram_tensor("embT", [P, T * P], f32, kind="ExternalInput")
    W1_d = nc.dram_tensor("W1", [EDIM, H], f32, kind="ExternalInput")
    W2_d = nc.dram_tensor("W2", [H, H], f32, kind="ExternalInput")
    b1_d = nc.dram_tensor("b1r", [1, H], f32, kind="ExternalInput")
    b2_d = nc.dram_tensor("b2r", [1, H], f32, kind="ExternalInput")
    dinv_d = nc.dram_tensor("dinv_t", [P, T], f32, kind="ExternalInput")
    sqd_d = nc.dram_tensor("sqd_row", [1, T * P], bf16, kind="ExternalInput")
    idx_d = [nc.dram_tensor(f"idx{s}", [P, max(int(S_s[s]), 1) * 8], i16,
                            kind="ExternalInput") for s in range(NSTR)]
    dst_d = nc.dram_tensor("dstloc", [P, max(total_chunks, 1)], bf16,
                           kind="ExternalInput")
    out_d = nc.dram_tensor("out", [npc, H], f32, kind="ExternalOutput")

    with tile.TileContext(nc) as tc:
        with tc.tile_pool(name="const", bufs=1) as constp, \
             tc.tile_pool(name="tables", bufs=1) as tablep, \
             tc.tile_pool(name="work", bufs=3) as workp, \
             tc.tile_pool(name="gath", bufs=6) as gathp, \
             tc.tile_pool(name="gathb", bufs=8) as gathbp, \
             tc.tile_pool(name="spool", bufs=3) as spool, \
             tc.tile_pool(name="psum", bufs=4, space="PSUM") as psump, \
             tc.tile_pool(name="psumT", bufs=2, space="PSUM") as psumTp, \
             tc.tile_pool(name="dram", bufs=1, space="DRAM") as dramp:

            # ---- constants ----
            from concourse.masks import make_identity
            ident = constp.tile([P, P], bf16)
            make_identity(nc, ident[:])
            ident_f = constp.tile([P, P], f32)
            make_identity(nc, ident_f[:])
            iota_i = constp.tile([P, SBATCH * P], i32)
            nc.gpsimd.iota(iota_i[:], pattern=[[0, SBATCH], [1, P]],
                           base=0, channel_multiplier=0)
            iota_f = constp.tile([P, SBATCH * P], bf16)
            nc.vector.tensor_copy(iota_f[:], iota_i[:])

            W1_s = constp.tile([EDIM, H], f32)
            nc.sync.dma_start(out=W1_s[:], in_=W1_d[:])
            W2_s = constp.tile([H, H], f32)
            nc.sync.dma_start(out=W2_s[:], in_=W2_d[:])
            b1_s = constp.tile([1, H], f32)
            nc.sync.dma_start(out=b1_s[:], in_=b1_d[:])
            b2_s = constp.tile([1, H], f32)
            nc.sync.dma_start(out=b2_s[:], in_=b2_d[:])
            dinv_s = constp.tile([P, T], f32)
            nc.sync.dma_start(out=dinv_s[:], in_=dinv_d[:])
            sqd_b = constp.tile([1, T * P], bf16)
            nc.sync.dma_start(out=sqd_b[:], in_=sqd_d[:])
            # bf16 copies for the scatter-path matmuls
            b1_b = constp.tile([1, H], bf16)
            nc.scalar.copy(b1_b[:], b1_s[:])
            b2_b = constp.tile([1, H], bf16)
            nc.scalar.copy(b2_b[:], b2_s[:])

            # ---- persistent tables in SBUF ----
            y1_all = tablep.tile([P, T * H], f32)     # layer-1 table, own slice
            y2_all = tablep.tile([P, T * H], f32)     # layer-2 table, own slice
            # idx streams + dstloc are identical for both layers and small:
            # keep them resident so gathers never wait on per-group DMAs
            maxS = max(int(x) for x in S_s)
            idx_all = tablep.tile([P, max(maxS, 1) * 8], i16)
            for s in range(NSTR):
                if int(S_s[s]) > 0:
                    nc.sync.dma_start(
                        out=idx_all[32 * s:32 * s + 32, :int(S_s[s]) * 8],
                        in_=idx_d[s][32 * s:32 * s + 32, :])
            dst_all = tablep.tile([P, max(total_chunks, 1)], bf16)
            nc.sync.dma_start(out=dst_all[:], in_=dst_d[:])

            # ---- DRAM staging for collectives ----
            y1_in = dramp.tile([npc, H], f32)
            y2_in = dramp.tile([npc, H], f32)
            # Shared pair-HBM outputs let the AllGather skip per-core copies
            y1_full = nc.dram_tensor("y1_full_sh", [N, H], f32,
                                     addr_space="Shared")
            y2_full = nc.dram_tensor("y2_full_sh", [N, H], f32,
                                     addr_space="Shared")

            TH = (T * 3) // 4        # tiles in the early AllGather half
            # ---- phase 1: y1 = dinv * (emb @ W1) for own nodes ----
            ECH = 16                       # emb tiles per DMA batch
            for t0 in range(0, T, ECH):
                ntile = min(ECH, T - t0)
                xt = workp.tile([P, ECH * P], f32, tag="embT")
                nc.sync.dma_start(out=xt[:, :ntile * P],
                                  in_=embT[:, t0 * P:(t0 + ntile) * P])
                for dt_i in range(ntile):
                    t = t0 + dt_i
                    ps = psump.tile([P, H], f32, tag="ps")
                    nc.tensor.matmul(ps[:], lhsT=xt[:, dt_i * P:(dt_i + 1) * P],
                                     rhs=W1_s[:], start=True, stop=True)
                    ys = y1_all[:, t * H:(t + 1) * H]
                    nc.vector.tensor_scalar_mul(ys, ps[:], dinv_s[:, t:t + 1])
                    rows = min(npc - t * P, P)
                    nc.sync.dma_start(out=y1_in[t * P:t * P + rows, :],
                                      in_=y1_all[:rows, t * H:(t + 1) * H])

            # ---- phase 2: AllGather layer-1 table ----
            nc.gpsimd.collective_compute(
                "AllGather", mybir.AluOpType.bypass,
                replica_groups=[list(range(n_cores))],
                ins=[y1_in.opt()],
                outs=[y1_full[:, :].opt()],
            )

            # ---- aggregation pass (used for both layers) ----
            def aggregation(src_table, y_own, b_s, layer):
                # per-stream gather bookkeeping
                next_group = [0] * NSTR       # next gather group to issue
                group_start = [0] * NSTR      # chunk index where current group starts
                gbufs = [None] * NSTR
                sbuf_tile = [None]             # current one-hot batch tile
                sbatch_lo = [-1]
                prefix = [np.concatenate([[0], np.cumsum(gather_groups[s])])
                          .astype(int) for s in range(NSTR)]

                def ensure_gather(b, pos):
                    while gbufs[b] is None or pos >= group_start[b] + gbufs[b][1]:
                        g = next_group[b]
                        start = int(prefix[b][g])
                        size = gather_groups[b][g]
                        gt = gathp.tile([P, GCHUNK, H], f32, tag=f"g{b}")
                        base = (b // 2) * BS + ABASE
                        nc.gpsimd.dma_gather(
                            out_ap=gt[:, :size, :],
                            in_ap=src_table[base:min(base + ABASE, N), :],
                            idxs_ap=idx_all[:, start * 8:(start + size) * 8],
                            num_idxs=size * P,
                            num_idxs_reg=size * P,
                            elem_size=H,
                            queue_num=b,
                        )
                        gtb = gathbp.tile([P, GCHUNK, H], bf16, tag=f"gb{b}")
                        nc.scalar.copy(gtb[:, :size, :], gt[:, :size, :])
                        gbufs[b] = (gtb, size)
                        group_start[b] = start
                        next_group[b] += 1
                    return gbufs[b][0][:, pos - group_start[b], :]

                def ensure_s(m):
                    lo = (m // SBATCH) * SBATCH
                    if sbatch_lo[0] != lo:
                        nb = min(SBATCH, total_chunks - lo)
                        st = spool.tile([P, SBATCH * P], bf16, tag="S")
                        dl = dst_all[:, lo:lo + nb]
                        dl3 = dl.rearrange("p (c u) -> p c u", u=1)
                        nc.vector.tensor_tensor(
                            out=st[:, :nb * P].rearrange("p (c j) -> p c j", j=P),
                            in0=iota_f[:, :nb * P].rearrange("p (c j) -> p c j", j=P),
                            in1=dl3.to_broadcast([P, nb, P]),
                            op=mybir.AluOpType.is_equal)
                        sbuf_tile[0] = st
                        sbatch_lo[0] = lo
                    return sbuf_tile[0][:, (m - sbatch_lo[0]) * P:
                                        (m - sbatch_lo[0] + 1) * P]

                for t in range(T):
                    ps = psump.tile([P, H], f32, tag="ps")
                    first = True
                    for b in range(BLOCKS):
                        for k in range(int(K[t, b])):
                            s_ = 2 * b + ((k + t) & 1)
                            pos = int(pos_s[t, s_]) + k // 2
                            m = int(m_start[t, b]) + k
                            gview = ensure_gather(s_, pos)
                            sview = ensure_s(m)
                            nc.tensor.matmul(ps[:], lhsT=sview, rhs=gview,
                                             start=first, stop=False)
                            first = False
                    # self-loop: psum += y_own tile (cast to bf16 for the PE)
                    yb = workp.tile([P, H], bf16, tag="yb")
                    nc.scalar.copy(yb[:], y_own[:, t * H:(t + 1) * H])
                    nc.tensor.matmul(ps[:], lhsT=ident[:],
                                     rhs=yb[:],
                                     start=first, stop=not has_bias)
                    if has_bias:
                        # bias premultiplied by sqrt(deg): += sqd_j * b_d
                        nc.tensor.matmul(ps[:], lhsT=sqd_b[:, t * P:(t + 1) * P],
                                         rhs=b_s[:], start=False, stop=True)
                    yield t, ps

            # ---- phase 3: layer-1 aggregation + fused layer-2 table ----
            for t, ps in aggregation(y1_full, y1_all, b1_b, 1):
                h1 = workp.tile([P, H], f32, tag="h1")
                nc.scalar.activation(h1[:], ps[:],
                                     mybir.ActivationFunctionType.Relu,
                                     scale=dinv_s[:, t:t + 1])
                pT = psumTp.tile([H, P], f32)
                nc.tensor.transpose(pT[:], h1[:], ident_f[:])
                h1T = workp.tile([H, P], f32, tag="h1T")
                nc.vector.tensor_copy(h1T[:], pT[:])
                ps2 = psump.tile([P, H], f32, tag="ps")
                nc.tensor.matmul(ps2[:], lhsT=h1T[:], rhs=W2_s[:],
                                 start=True, stop=True)
                y2s = y2_all[:, t * H:(t + 1) * H]
                nc.vector.tensor_scalar_mul(y2s, ps2[:], dinv_s[:, t:t + 1])
                rows = min(npc - t * P, P)
                nc.sync.dma_start(out=y2_in[t * P:t * P + rows, :],
                                  in_=y2_all[:rows, t * H:(t + 1) * H])

            # ---- phase 4: AllGather layer-2 table ----
            nc.gpsimd.collective_compute(
                "AllGather", mybir.AluOpType.bypass,
                replica_groups=[list(range(n_cores))],
                ins=[y2_in.opt()],
                outs=[y2_full[:, :].opt()],
            )

            # ---- phase 5: layer-2 aggregation -> output ----
            for t, ps in aggregation(y2_full, y2_all, b2_b, 2):
                ot = workp.tile([P, H], f32, tag="ot")
                nc.vector.tensor_scalar_mul(ot[:], ps[:], dinv_s[:, t:t + 1])
                rows = min(npc - t * P, P)
                nc.sync.dma_start(out=out_d[t * P:t * P + rows, :],
                                  in_=ot[:rows, :])

    nc.compile()
    return nc


def make_inputs(sched, emb_weight, W1, b1, W2, b2, deg):
    """Build per-core input maps."""
    n_cores = sched["n_cores"]
    npc = sched["npc"]
    T = sched["T"]
    dinv = (1.0 / np.sqrt(deg.astype(np.float64))).astype(np.float32)
    sqd = np.sqrt(deg.astype(np.float64)).astype(np.float32)
    in_maps = []
    for c in range(n_cores):
        lo, hi = c * npc, (c + 1) * npc
        embT = np.zeros((P, T * P), np.float32)
        embT[:, :npc] = emb_weight[lo:hi].T
        tmp = np.zeros(T * P, np.float32)
        tmp[:npc] = dinv[lo:hi]
        dinv_t = np.ascontiguousarray(tmp.reshape(T, P).T)
        sqd_row = np.zeros((1, T * P), np.float32)
        sqd_row[0, :npc] = sqd[lo:hi]
        idx_wrapped, dst_t = sched["core_data"][c]
        m = {
            "embT": embT,
            "W1": W1.astype(np.float32),
            "W2": W2.astype(np.float32),
            "b1r": b1.reshape(1, -1).astype(np.float32),
            "b2r": b2.reshape(1, -1).astype(np.float32),
            "dinv_t": dinv_t,
            "sqd_row": sqd_row.astype(ml_dtypes.bfloat16),
            "dstloc": dst_t.astype(ml_dtypes.bfloat16),
        }
        for s in range(NSTR):
            iw = idx_wrapped[s]
            if iw.shape[1] == 0:
                iw = np.zeros((P, 8), np.int16)
            m[f"idx{s}"] = iw
        in_maps.append(m)
    return in_maps


def run(edge_index, emb_weight, W1, b1, W2, b2, n_nodes=None, trace=False):
    from concourse import bass_utils
    n_nodes = n_nodes if n_nodes is not None else emb_weight.shape[0]
    sched = make_schedule(np.asarray(edge_index), n_nodes)
    has_bias = bool(np.any(np.asarray(b1)) or np.any(np.asarray(b2)))
    key = ("gnn", n_nodes, int(sched["total_chunks"]), has_bias,
           tuple(int(x) for x in sched["S_s"]))
    if key not in _COMPILED:
        _COMPILED[key] = build_bass(sched, has_bias)
    nc = _COMPILED[key]
    deg = np.bincount(np.asarray(edge_index)[1], minlength=n_nodes).astype(np.float32) + 1.0
    in_maps = make_inputs(sched, np.asarray(emb_weight), np.asarray(W1),
                          np.asarray(b1), np.asarray(W2), np.asarray(b2), deg)
    res = bass_utils.run_bass_kernel_spmd(
        nc, in_maps, core_ids=list(range(sched["n_cores"])), trace=trace)
    npc = sched["npc"]
    out = np.concatenate([res.results[c]["out"] for c in range(sched["n_cores"])],
                         axis=0)
    return out[:n_nodes], res


def kernel(edge_index, emb_weight, W1, b1, W2, b2):
    out, _ = run(edge_index, emb_weight, W1, b1, W2, b2)
    return out

